# revision 13
# baseline (speedup 1.0000x reference)
"""Trainium2 Bass kernel for a GroupNorm->QKV->MHA->proj->residual block.

Problem shapes (hardcoded): x [16, 512, 32, 32] fp32, 4 heads, head_dim 128,
32 GN groups. Sharding: data-parallel over batch across 8 NeuronCores
(2 items per core), no collectives.

v2: fp8 (TRN float8e4, max 240) DoubleRow matmuls for QKV / PV / softmax
denominator / proj (2x PE rate via 256-deep contraction pairs); QK^T stays
fp16 (d=128 contraction cannot use DoubleRow). Weights are scaled x16 on
host so fp8 quantization stays out of the subnormal range; the scale is
repaid inside the exp activation (scale=SCALE/256) and the final residual
affine (1/256). The softmax denominator is computed on the PE with an fp8
ones-matmul over the exp chunks (replaces ~36us of DVE tree adds).

Layout per core, per item:
  x      [c=512, n=1024]  as 4 c-tiles of [128, 1024] fp32
  h8     = GN(x) in fp8, [128, CT, HW] (QKV DoubleRow rhs)
  Q',K'  [d=128, n=1024] fp16 per head, = 16*(Q,K) (plain psum copy evac)
  V'^T   [m=1024, d=512] fp8, = 16*V^T via swapped-operand DR matmul
  S'     = K'^T Q' fp16 matmul = 256*S; exp scale folds 1/256
  E^T    = exp(S*scale - 1.5) fp8 [128, MT, HW] (max ~134 < 240)
  denom  = ones8-DR-matmul over E^T chunks, accumulated in PSUM [128, HW]
  U'     = V'^T.T @ E^T fp8-DR in PSUM; O' = U' * (1/denom) in fp8 (=16*O)
  proj   fp8-DR PSUM matmul (=256*proj), then one DVE affine_then_add:
         out = (psum*(1/256) + proj_b') + x
  proj_b' (host) = proj_b + proj_w @ qkv_b[1024:]  (folds the V bias exactly)
"""

import os
import numpy as np
import ml_dtypes
from contextlib import ExitStack

from concourse import bass, bacc, mybir
import concourse.tile as tile
from concourse import bass_utils

FP32 = mybir.dt.float32
FP16 = mybir.dt.float16
FP8 = mybir.dt.float8e4
NP8 = ml_dtypes.float8_e4m3

N_CORES = 8
B = 16
ITEMS = B // N_CORES      # batch items per core
C = 512
HW = 1024                 # H*W
HEADS = 4
D = 128                   # head dim
CT = C // 128             # channel tiles
MT = HW // 128            # m-chunks of the softmax axis
NHALF = 512               # matmul free-dim half (one PSUM bank)
SCALE = float(D) ** -0.5
WS = 16.0                 # host weight scale (fp8 subnormal dodge)
EXP_BIAS = -1.5
EPS = 1e-5
DR = mybir.MatmulPerfMode.DoubleRow

_CACHE = {}


def _build_bass():
    nc = bacc.Bacc()

    x_d = nc.declare_dram_parameter("x", [ITEMS, C, HW], FP32, isOutput=False)
    qkvw_d = nc.declare_dram_parameter("qkv_wT", [C, 3 * C], FP8, isOutput=False)
    qkvb_d = nc.declare_dram_parameter("qkv_bt", [128, 8], FP32, isOutput=False)
    projw_d = nc.declare_dram_parameter("proj_wT", [C, C], FP8, isOutput=False)
    projb_d = nc.declare_dram_parameter("proj_bt", [128, CT], FP32, isOutput=False)
    gnw_d = nc.declare_dram_parameter("gn_wt", [128, CT], FP32, isOutput=False)
    gnb_d = nc.declare_dram_parameter("gn_bt", [128, CT], FP32, isOutput=False)
    inda_d = nc.declare_dram_parameter("ind_a", [128, 8], FP32, isOutput=False)
    indb_d = nc.declare_dram_parameter("ind_b", [8, 128], FP32, isOutput=False)
    out_d = nc.declare_dram_parameter("out", [ITEMS, C, HW], FP32, isOutput=True)

    AF = mybir.ActivationFunctionType
    ALU = mybir.AluOpType

    with ExitStack() as ctx:
        tc = ctx.enter_context(tile.TileContext(nc))
        singles = ctx.enter_context(tc.tile_pool(name="singles", bufs=1))
        xp = ctx.enter_context(tc.tile_pool(name="xp", bufs=2))
        hp = ctx.enter_context(tc.tile_pool(name="hp", bufs=2))
        qkp = ctx.enter_context(tc.tile_pool(name="qkp", bufs=2))
        vtp = ctx.enter_context(tc.tile_pool(name="vtp", bufs=2))
        etp = ctx.enter_context(tc.tile_pool(name="etp", bufs=2))
        rip = ctx.enter_context(tc.tile_pool(name="rip", bufs=2))
        oap = ctx.enter_context(tc.tile_pool(name="oap", bufs=2))
        oup = ctx.enter_context(tc.tile_pool(name="oup", bufs=2))
        stp = ctx.enter_context(tc.tile_pool(name="stp", bufs=4))
        psp = ctx.enter_context(tc.tile_pool(name="psp", bufs=2, space="PSUM"))

        # ---- item-0 x first (startup critical path), then weights ----
        x_sbs = {}
        x_sbs[0] = xp.tile([128, CT, HW], FP32, name="x_0", tag="x")
        for t in range(CT):
            for sg in range(2):
                eng = nc.sync if (2 * t + sg) % 2 == 0 else nc.gpsimd
                eng.dma_start(out=x_sbs[0][:, t, sg * 512:(sg + 1) * 512],
                              in_=x_d[0, t * 128:(t + 1) * 128, sg * 512:(sg + 1) * 512])

        qkvw_sb = singles.tile([128, CT, 3 * C], FP8)
        for t in range(CT):
            eng = nc.sync if t % 2 == 0 else nc.gpsimd
            eng.dma_start(out=qkvw_sb[:, t, :], in_=qkvw_d[t * 128:(t + 1) * 128, :])
        # keep PE busy/warm during the GroupNorm startup window (HAM clock gate)
        warm_ps = psp.tile([128, NHALF], FP32, name="warm_ps", tag="st")
        for _ in range(16):
            nc.tensor.matmul(out=warm_ps, lhsT=qkvw_sb[:, 0, 0:128],
                             rhs=qkvw_sb[:, 0, 0:NHALF], start=True, stop=True)
        projw_sb = singles.tile([128, CT, C], FP8)
        for t in range(CT):
            eng = nc.sync if t % 2 == 0 else nc.gpsimd
            eng.dma_start(out=projw_sb[:, t, :], in_=projw_d[t * 128:(t + 1) * 128, :])
        qkvb_sb = singles.tile([128, 8], FP32)
        nc.sync.dma_start(out=qkvb_sb, in_=qkvb_d[:, :])
        projb_sb = singles.tile([128, CT], FP32)
        nc.sync.dma_start(out=projb_sb, in_=projb_d[:, :])
        gnw_sb = singles.tile([128, CT], FP32)
        nc.gpsimd.dma_start(out=gnw_sb, in_=gnw_d[:, :])
        gnb_sb = singles.tile([128, CT], FP32)
        nc.gpsimd.dma_start(out=gnb_sb, in_=gnb_d[:, :])
        inda_sb = singles.tile([128, 8], FP32)
        nc.sync.dma_start(out=inda_sb, in_=inda_d[:, :])
        indb_sb = singles.tile([8, 128], FP32)
        nc.gpsimd.dma_start(out=indb_sb, in_=indb_d[:, :])
        ones_sb = singles.tile([128, 2, 128], FP8)
        nc.vector.memset(ones_sb, 1.0)
        eps_sb = singles.tile([128, 1], FP32)
        nc.vector.memset(eps_sb, EPS)
        ebias_sb = singles.tile([128, 1], FP32)
        nc.vector.memset(ebias_sb, EXP_BIAS)
        warm_sb = singles.tile([128, 1], FP32)
        nc.vector.memset(warm_sb, 1.0)
        nc.scalar.activation(out=warm_sb, in_=warm_sb, func=AF.Sqrt,
                             bias=eps_sb[:, 0:1], scale=1.0)

        SGD = nc.vector.BN_STATS_DIM   # 6
        NSUB = HW // nc.vector.BN_STATS_FMAX  # 2 subgroups of 512

        def emit_x_dma(it):
            x_sb = xp.tile([128, CT, HW], FP32, name=f"x_{it}", tag="x")
            x_sbs[it] = x_sb
            for t in range(CT):
                for sg in range(2):
                    eng = nc.sync if (2 * t + sg) % 2 == 0 else nc.gpsimd
                    eng.dma_start(out=x_sb[:, t, sg * 512:(sg + 1) * 512],
                                  in_=x_d[it, t * 128:(t + 1) * 128, sg * 512:(sg + 1) * 512])

        def emit_gn(it):
            """Batched GroupNorm: one combine chain for all 4 c-tiles.
            Writes h in fp8 (QKV DoubleRow rhs)."""
            x_sb = x_sbs[it]
            h_sb = hp.tile([128, CT, HW], FP8, name=f"h_{it}", tag="h")
            stats128 = stp.tile([128, 2 * CT], FP32, name=f"s128_{it}", tag="s128")
            mvs = stp.tile([128, CT, 2], FP32, name=f"mvs_{it}", tag="mvs")
            for t in range(CT):
                bnst = stp.tile([128, NSUB, SGD], FP32, name=f"bnst_{it}_{t}", tag="bnst", bufs=2)
                for sg in range(NSUB):
                    nc.vector.bn_stats(out=bnst[:, sg, :], in_=x_sb[:, t, sg * 512:(sg + 1) * 512])
                nc.vector.bn_aggr(out=mvs[:, t, :], in_=bnst)
            s1v = stats128.rearrange("p (t two) -> p t two", two=2)
            nc.vector.tensor_copy(out=s1v[:, :, 0], in_=mvs[:, :, 0])
            nc.vector.tensor_mul(out=s1v[:, :, 1], in0=mvs[:, :, 0], in1=mvs[:, :, 0])
            nc.vector.tensor_add(out=s1v[:, :, 1], in0=s1v[:, :, 1], in1=mvs[:, :, 1])
            gst_ps = psp.tile([8, 2 * CT], FP32, name=f"gstp_{it}", tag="st")
            nc.tensor.matmul(out=gst_ps, lhsT=inda_sb, rhs=stats128, start=True, stop=True)
            gst_sb = stp.tile([8, 2 * CT], FP32, name=f"gst_{it}", tag="gst")
            nc.vector.tensor_copy(out=gst_sb, in_=gst_ps)
            gv = gst_sb.rearrange("p (t two) -> p t two", two=2)
            gm2 = stp.tile([8, CT], FP32, name=f"gm2_{it}", tag="gm2")
            nc.vector.tensor_mul(out=gm2, in0=gv[:, :, 0], in1=gv[:, :, 0])
            nc.vector.tensor_tensor(out=gm2, in0=gv[:, :, 1], in1=gm2, op=ALU.subtract)
            nc.scalar.activation(out=gm2, in_=gm2, func=AF.Sqrt,
                                 bias=eps_sb[0:8, 0:1], scale=1.0)
            nc.vector.reciprocal(out=gv[:, :, 1], in_=gm2)
            chst_ps = psp.tile([128, 2 * CT], FP32, name=f"chstp_{it}", tag="st")
            nc.tensor.matmul(out=chst_ps, lhsT=indb_sb, rhs=gst_sb, start=True, stop=True)
            chst_sb = stp.tile([128, 2 * CT], FP32, name=f"chst_{it}", tag="chst")
            nc.vector.tensor_copy(out=chst_sb, in_=chst_ps)
            cv = chst_sb.rearrange("p (t two) -> p t two", two=2)
            sb2 = stp.tile([128, 2, CT], FP32, name=f"sb2_{it}", tag="sb2")
            nc.vector.tensor_mul(out=sb2[:, 0, :], in0=cv[:, :, 1], in1=gnw_sb)
            nc.vector.tensor_mul(out=sb2[:, 1, :], in0=cv[:, :, 0], in1=sb2[:, 0, :])
            nc.vector.tensor_tensor(out=sb2[:, 1, :], in0=gnb_sb, in1=sb2[:, 1, :], op=ALU.subtract)
            # item 0's apply is on the startup critical path -> fast DVE;
            # item 1's goes to the otherwise-idle gpsimd engine
            aeng = nc.vector if it == 0 else nc.gpsimd
            for t in range(CT):
                aeng.tensor_scalar(
                    out=h_sb[:, t, :], in0=x_sb[:, t, :],
                    scalar1=sb2[:, 0, t:t + 1], scalar2=sb2[:, 1, t:t + 1],
                    op0=ALU.mult, op1=ALU.add)
            return h_sb

        def emit_qk_pair(it, qk_sb, h_sb, hd):
            """Q'(=16Q) and K'(=16K) for one head via fp8 DoubleRow QKV matmul;
            evac q on scalar engine, k on vector engine (engine balance)."""
            for ot in (hd, 4 + hd):
                qk_ps = psp.tile([128, HW], FP32, name=f"qkps_{it}_{ot}", tag="st")
                for j in range(2):           # contraction pairs (stationary reuse over halves)
                    for half in range(2):
                        nc.tensor.matmul(
                            out=qk_ps[:, half * NHALF:(half + 1) * NHALF],
                            lhsT=qkvw_sb[:, 2 * j:2 * j + 2, ot * 128:(ot + 1) * 128],
                            rhs=h_sb[:, 2 * j:2 * j + 2, half * NHALF:(half + 1) * NHALF],
                            start=(j == 0), stop=(j == 1), perf_mode=DR)
                # balance evacuations: alternate which of q/k goes to ACT vs DVE
                on_act = (ot < 4) == (hd % 2 == 0)
                if on_act:
                    nc.scalar.activation(out=qk_sb[:, ot, :], in_=qk_ps,
                                         func=AF.Identity,
                                         bias=qkvb_sb[:, ot:ot + 1], scale=1.0)
                else:
                    nc.vector.tensor_scalar_add(
                        out=qk_sb[:, ot, :], in0=qk_ps, scalar1=qkvb_sb[:, ot:ot + 1])

        def emit_vt(it, vt_sb, h_sb, mt_pairs):
            """V'^T (=16 V^T) [m, d-all-heads] fp8 via swapped-operand DR matmul.
            Two m-chunks share one PSUM tile so the fp8 cast is one DVE op."""
            for mp in mt_pairs:
                vt_ps = psp.tile([128, 2, C], FP32, name=f"vtps_{it}_{mp}", tag="st")
                for half in range(2):
                    mt = 2 * mp + half
                    for j in range(2):
                        nc.tensor.matmul(
                            out=vt_ps[:, half, :],
                            lhsT=h_sb[:, 2 * j:2 * j + 2, mt * 128:(mt + 1) * 128],
                            rhs=qkvw_sb[:, 2 * j:2 * j + 2, 2 * C:3 * C],
                            start=(j == 0), stop=(j == 1), perf_mode=DR)
                nc.vector.tensor_copy(out=vt_sb[:, 2 * mp:2 * mp + 2, :], in_=vt_ps)

        def emit_proj_ot(it, o_sb, ot, via_act=False):
            pr_ps = psp.tile([128, HW], FP32, name=f"prps_{it}_{ot}", tag="st")
            for j in range(2):
                for half in range(2):
                    nc.tensor.matmul(
                        out=pr_ps[:, half * NHALF:(half + 1) * NHALF],
                        lhsT=projw_sb[:, 2 * j:2 * j + 2, ot * 128:(ot + 1) * 128],
                        rhs=o_sb[:, 2 * j:2 * j + 2, half * NHALF:(half + 1) * NHALF],
                        start=(j == 0), stop=(j == 1), perf_mode=DR)
            out_sb = oup.tile([128, HW], FP32, name=f"out_{it}_{ot}", tag="outsb")
            if via_act:
                # kernel tail: ACT is done with exp, DVE is the bottleneck there
                tmp_sb = oup.tile([128, HW], FP32, name=f"tmp_{it}_{ot}", tag="tmpsb", bufs=2)
                nc.scalar.activation(out=tmp_sb, in_=pr_ps, func=AF.Identity,
                                     bias=projb_sb[:, ot:ot + 1], scale=1.0 / (WS * WS))
                nc.gpsimd.tensor_add(out=out_sb, in0=tmp_sb, in1=x_sbs[it][:, ot, :])
            else:
                nc.vector.affine_then_add(
                    out=out_sb, in0=pr_ps, in1=x_sbs[it][:, ot, :],
                    scale=1.0 / (WS * WS), bias=projb_sb[:, ot:ot + 1])
            (nc.sync if ot % 2 == 0 else nc.gpsimd).dma_start(
                out=out_d[it, ot * 128:(ot + 1) * 128, :], in_=out_sb)

        def emit_attn(it, qk_sb, vt_sb, inject=None):
            """Attention; denominator via fp8 ones-DR-matmul in PSUM, one-head
            deferred flush; `inject` maps head idx -> filler work after PV."""
            o_sb = oap.tile([128, CT, HW], FP8, name=f"o_{it}", tag="o")
            pend = None

            def flush(p):
                hd_p, db_p, u_p = p
                rinv = rip.tile([128, HW], FP32, name=f"rinv_{it}_{hd_p}", tag="rinv")
                nc.vector.reciprocal_approx_fast(out=rinv, in_=db_p)
                nc.vector.tensor_mul(out=o_sb[:, hd_p, :], in0=u_p, in1=rinv)

            for hd in range(HEADS):
                if pend is not None:
                    flush(pend)
                    pend = None
                eT = etp.tile([128, MT, HW], FP8, name=f"eT_{it}_{hd}", tag="eT")
                for mt in range(MT):
                    st_ps = psp.tile([128, HW], FP32, name=f"stps_{it}_{hd}_{mt}", tag="st")
                    for half in range(2):
                        nc.tensor.matmul(
                            out=st_ps[:, half * NHALF:(half + 1) * NHALF],
                            lhsT=qk_sb[:, 4 + hd, mt * 128:(mt + 1) * 128],
                            rhs=qk_sb[:, hd, half * NHALF:(half + 1) * NHALF],
                            start=True, stop=True)
                    nc.scalar.activation(out=eT[:, mt, :], in_=st_ps,
                                         func=AF.Exp, bias=ebias_sb,
                                         scale=SCALE / (WS * WS))

                # softmax denominator: ones-DR-matmul over the 8 m-chunks,
                # broadcast to all 128 partitions in PSUM
                db_ps = psp.tile([128, HW], FP32, name=f"dbps_{it}_{hd}", tag="db", bufs=1)
                for half in range(2):
                    for j in range(MT // 2):
                        nc.tensor.matmul(
                            out=db_ps[:, half * NHALF:(half + 1) * NHALF],
                            lhsT=ones_sb,
                            rhs=eT[:, 2 * j:2 * j + 2, half * NHALF:(half + 1) * NHALF],
                            start=(j == 0), stop=(j == MT // 2 - 1), perf_mode=DR)

                if inject is not None and hd in inject:
                    inject[hd]()
                u_ps = psp.tile([128, HW], FP32, name=f"ups_{it}_{hd}", tag="u", bufs=1)
                for j in range(MT // 2):     # stationary reuse over halves
                    for half in range(2):
                        nc.tensor.matmul(
                            out=u_ps[:, half * NHALF:(half + 1) * NHALF],
                            lhsT=vt_sb[:, 2 * j:2 * j + 2, hd * 128:(hd + 1) * 128],
                            rhs=eT[:, 2 * j:2 * j + 2, half * NHALF:(half + 1) * NHALF],
                            start=(j == 0), stop=(j == MT // 2 - 1), perf_mode=DR)
                pend = (hd, db_ps, u_ps)
            flush(pend)
            return o_sb

        # ---------- schedule ----------
        h0 = emit_gn(0)
        qk0 = qkp.tile([128, 8, HW], FP16, name="qk_0", tag="qk")
        for hd in range(HEADS):
            emit_qk_pair(0, qk0, h0, hd)
        vt0 = vtp.tile([128, MT, C], FP8, name="vt_0", tag="vt")
        emit_vt(0, vt0, h0, range(MT // 2))
        emit_x_dma(1)
        h1 = emit_gn(1)

        # attention(0) with item-1 QKV interleaved between heads
        qk1 = qkp.tile([128, 8, HW], FP16, name="qk_1", tag="qk")
        vt1 = vtp.tile([128, MT, C], FP8, name="vt_1", tag="vt")
        inj0 = {
            0: lambda: emit_qk_pair(1, qk1, h1, 0),
            1: lambda: (emit_qk_pair(1, qk1, h1, 1), emit_qk_pair(1, qk1, h1, 2)),
            2: lambda: (emit_qk_pair(1, qk1, h1, 3), emit_vt(1, vt1, h1, range(0, 2))),
            3: lambda: emit_vt(1, vt1, h1, range(2, MT // 2)),
        }
        o0 = emit_attn(0, qk0, vt0, inject=inj0)

        # attention(1) with item-0 proj interleaved between heads
        inj1 = {
            1: lambda: (emit_proj_ot(0, o0, 0), emit_proj_ot(0, o0, 1)),
            2: lambda: emit_proj_ot(0, o0, 2),
            3: lambda: emit_proj_ot(0, o0, 3),
        }
        o1 = emit_attn(1, qk1, vt1, inject=inj1)
        for ot in range(CT):
            emit_proj_ot(1, o1, ot, via_act=(ot % 2 == 0))

    nc.compile()
    return nc


def _host_prep(x, gn_w, gn_b, qkv_w, qkv_b, proj_w, proj_b):
    x = np.ascontiguousarray(np.asarray(x, dtype=np.float32)).reshape(B, C, HW)
    qkv_w = np.asarray(qkv_w, dtype=np.float32)
    qkv_b = np.asarray(qkv_b, dtype=np.float32)
    proj_w = np.asarray(proj_w, dtype=np.float32)
    proj_b = np.asarray(proj_b, dtype=np.float32)
    gn_w = np.asarray(gn_w, dtype=np.float32)
    gn_b = np.asarray(gn_b, dtype=np.float32)

    qkv_wT = np.ascontiguousarray(qkv_w.T * WS).astype(NP8)            # [C, 3C]
    proj_wT = np.ascontiguousarray(proj_w.T * WS).astype(NP8)          # [C, C]
    # q,k evac adds 16*bias (Q' = 16Q + 16 b_q)
    qkv_bt = np.ascontiguousarray((WS * qkv_b[:2 * C]).reshape(8, 128).T)  # [128, 8]
    proj_be = proj_b + proj_w @ qkv_b[2 * C:]                          # fold V bias
    proj_bt = np.ascontiguousarray(proj_be.reshape(CT, 128).T)         # [128, CT]
    gn_wt = np.ascontiguousarray(gn_w.reshape(CT, 128).T)
    gn_bt = np.ascontiguousarray(gn_b.reshape(CT, 128).T)

    p = np.arange(128)
    ind_a = np.zeros((128, 8), np.float32)
    ind_a[p, p // 16] = 1.0 / 16.0
    ind_b = np.zeros((8, 128), np.float32)
    ind_b[p // 16, p] = 1.0

    shared = {
        "qkv_wT": qkv_wT, "qkv_bt": qkv_bt.astype(np.float32),
        "proj_wT": proj_wT, "proj_bt": proj_bt.astype(np.float32),
        "gn_wt": gn_wt.astype(np.float32), "gn_bt": gn_bt.astype(np.float32),
        "ind_a": ind_a, "ind_b": ind_b,
    }
    in_maps = []
    for c in range(N_CORES):
        m = dict(shared)
        m["x"] = np.ascontiguousarray(x[c * ITEMS:(c + 1) * ITEMS])
        in_maps.append(m)
    return in_maps


def kernel(x, gn_w, gn_b, qkv_w, qkv_b, proj_w, proj_b):
    if "nc" not in _CACHE:
        _CACHE["nc"] = _build_bass()
    nc = _CACHE["nc"]
    in_maps = _host_prep(x, gn_w, gn_b, qkv_w, qkv_b, proj_w, proj_b)
    trace = bool(int(os.environ.get("KERNEL_TRACE", "0")))
    res = bass_utils.run_bass_kernel_spmd(
        nc, in_maps, core_ids=list(range(N_CORES)), trace=trace)
    _CACHE["last_results"] = res
    out = np.concatenate([r["out"] for r in res.results], axis=0)
    return out.reshape(B, C, 32, 32).astype(np.float32)


# revision 22
# speedup vs baseline: 1.0076x; 1.0076x over previous
"""Trainium2 Bass kernel for a GroupNorm->QKV->MHA->proj->residual block.

Problem shapes (hardcoded): x [16, 512, 32, 32] fp32, 4 heads, head_dim 128,
32 GN groups. Sharding: data-parallel over batch across 8 NeuronCores
(2 items per core), no collectives.

v2: fp8 (TRN float8e4, max 240) DoubleRow matmuls for QKV / PV / softmax
denominator / proj (2x PE rate via 256-deep contraction pairs); QK^T stays
fp16 (d=128 contraction cannot use DoubleRow). Weights are scaled x16 on
host so fp8 quantization stays out of the subnormal range; the scale is
repaid inside the exp activation (scale=SCALE/256) and the final residual
affine (1/256). The softmax denominator is computed on the PE with an fp8
ones-matmul over the exp chunks (replaces ~36us of DVE tree adds).

Layout per core, per item:
  x      [c=512, n=1024]  as 4 c-tiles of [128, 1024] fp32
  h8     = GN(x) in fp8, [128, CT, HW] (QKV DoubleRow rhs)
  Q',K'  [d=128, n=1024] fp16 per head, = 16*(Q,K) (plain psum copy evac)
  V'^T   [m=1024, d=512] fp8, = 16*V^T via swapped-operand DR matmul
  S'     = K'^T Q' fp16 matmul = 256*S; exp scale folds 1/256
  E^T    = exp(S*scale - 1.5) fp8 [128, MT, HW] (max ~134 < 240)
  denom  = ones8-DR-matmul over E^T chunks, accumulated in PSUM [128, HW]
  U'     = V'^T.T @ E^T fp8-DR in PSUM; O' = U' * (1/denom) in fp8 (=16*O)
  proj   fp8-DR PSUM matmul (=256*proj), then one DVE affine_then_add:
         out = (psum*(1/256) + proj_b') + x
  proj_b' (host) = proj_b + proj_w @ qkv_b[1024:]  (folds the V bias exactly)
"""

import os
import numpy as np
import ml_dtypes
from contextlib import ExitStack

from concourse import bass, bacc, mybir
import concourse.tile as tile
from concourse import bass_utils

FP32 = mybir.dt.float32
FP16 = mybir.dt.float16
FP8 = mybir.dt.float8e4
NP8 = ml_dtypes.float8_e4m3

N_CORES = 8
B = 16
ITEMS = B // N_CORES      # batch items per core
C = 512
HW = 1024                 # H*W
HEADS = 4
D = 128                   # head dim
CT = C // 128             # channel tiles
MT = HW // 128            # m-chunks of the softmax axis
NHALF = 512               # matmul free-dim half (one PSUM bank)
SCALE = float(D) ** -0.5
WS = 16.0                 # host weight scale (fp8 subnormal dodge)
EXP_BIAS = -1.5
EPS = 1e-5
DR = mybir.MatmulPerfMode.DoubleRow

_CACHE = {}


def _build_bass():
    nc = bacc.Bacc()

    x_d = nc.declare_dram_parameter("x", [ITEMS, C, HW], FP32, isOutput=False)
    qkvw_d = nc.declare_dram_parameter("qkv_wT", [C, 3 * C], FP8, isOutput=False)
    qkvb_d = nc.declare_dram_parameter("qkv_bt", [128, 8], FP32, isOutput=False)
    projw_d = nc.declare_dram_parameter("proj_wT", [C, C], FP8, isOutput=False)
    projb_d = nc.declare_dram_parameter("proj_bt", [128, CT], FP32, isOutput=False)
    gnw_d = nc.declare_dram_parameter("gn_wt", [128, CT], FP32, isOutput=False)
    gnb_d = nc.declare_dram_parameter("gn_bt", [128, CT], FP32, isOutput=False)
    inda_d = nc.declare_dram_parameter("ind_a", [128, 8], FP32, isOutput=False)
    indb_d = nc.declare_dram_parameter("ind_b", [8, 128], FP32, isOutput=False)
    out_d = nc.declare_dram_parameter("out", [ITEMS, C, HW], FP32, isOutput=True)

    AF = mybir.ActivationFunctionType
    ALU = mybir.AluOpType

    with ExitStack() as ctx:
        tc = ctx.enter_context(tile.TileContext(nc))
        singles = ctx.enter_context(tc.tile_pool(name="singles", bufs=1))
        xp = ctx.enter_context(tc.tile_pool(name="xp", bufs=2))
        hp = ctx.enter_context(tc.tile_pool(name="hp", bufs=2))
        qkp = ctx.enter_context(tc.tile_pool(name="qkp", bufs=2))
        vtp = ctx.enter_context(tc.tile_pool(name="vtp", bufs=2))
        etp = ctx.enter_context(tc.tile_pool(name="etp", bufs=2))
        rip = ctx.enter_context(tc.tile_pool(name="rip", bufs=2))
        oap = ctx.enter_context(tc.tile_pool(name="oap", bufs=2))
        oup = ctx.enter_context(tc.tile_pool(name="oup", bufs=2))
        stp = ctx.enter_context(tc.tile_pool(name="stp", bufs=4))
        psp = ctx.enter_context(tc.tile_pool(name="psp", bufs=2, space="PSUM"))

        # ---- item-0 x first (startup critical path), then weights ----
        x_sbs = {}
        x_sbs[0] = xp.tile([128, CT, HW], FP32, name="x_0", tag="x")
        for t in range(CT):
            for sg in range(2):
                eng = nc.sync if (2 * t + sg) % 2 == 0 else nc.gpsimd
                eng.dma_start(out=x_sbs[0][:, t, sg * 512:(sg + 1) * 512],
                              in_=x_d[0, t * 128:(t + 1) * 128, sg * 512:(sg + 1) * 512])

        qkvw_sb = singles.tile([128, CT, 3 * C], FP8)
        for t in range(CT):
            eng = nc.sync if t % 2 == 0 else nc.gpsimd
            eng.dma_start(out=qkvw_sb[:, t, :], in_=qkvw_d[t * 128:(t + 1) * 128, :])
        # keep PE busy/warm during the GroupNorm startup window (HAM clock gate)
        warm_ps = psp.tile([128, NHALF], FP32, name="warm_ps", tag="st")
        for _ in range(16):
            nc.tensor.matmul(out=warm_ps, lhsT=qkvw_sb[:, 0, 0:128],
                             rhs=qkvw_sb[:, 0, 0:NHALF], start=True, stop=True)
        projw_sb = singles.tile([128, CT, C], FP8)
        for t in range(CT):
            eng = nc.sync if t % 2 == 0 else nc.gpsimd
            eng.dma_start(out=projw_sb[:, t, :], in_=projw_d[t * 128:(t + 1) * 128, :])
        qkvb_sb = singles.tile([128, 8], FP32)
        nc.sync.dma_start(out=qkvb_sb, in_=qkvb_d[:, :])
        projb_sb = singles.tile([128, CT], FP32)
        nc.sync.dma_start(out=projb_sb, in_=projb_d[:, :])
        gnw_sb = singles.tile([128, CT], FP32)
        nc.gpsimd.dma_start(out=gnw_sb, in_=gnw_d[:, :])
        gnb_sb = singles.tile([128, CT], FP32)
        nc.gpsimd.dma_start(out=gnb_sb, in_=gnb_d[:, :])
        inda_sb = singles.tile([128, 8], FP32)
        nc.sync.dma_start(out=inda_sb, in_=inda_d[:, :])
        indb_sb = singles.tile([8, 128], FP32)
        nc.gpsimd.dma_start(out=indb_sb, in_=indb_d[:, :])
        ones_sb = singles.tile([128, 2, 128], FP8)
        nc.vector.memset(ones_sb, 1.0)
        eps_sb = singles.tile([128, 1], FP32)
        nc.vector.memset(eps_sb, EPS)
        ebias_sb = singles.tile([128, 1], FP32)
        nc.vector.memset(ebias_sb, EXP_BIAS)
        warm_sb = singles.tile([128, 1], FP32)
        nc.vector.memset(warm_sb, 1.0)
        nc.scalar.activation(out=warm_sb, in_=warm_sb, func=AF.Sqrt,
                             bias=eps_sb[:, 0:1], scale=1.0)

        SGD = nc.vector.BN_STATS_DIM   # 6
        NSUB = HW // nc.vector.BN_STATS_FMAX  # 2 subgroups of 512

        def emit_x_dma(it):
            x_sb = xp.tile([128, CT, HW], FP32, name=f"x_{it}", tag="x")
            x_sbs[it] = x_sb
            for t in range(CT):
                for sg in range(2):
                    eng = nc.sync if (2 * t + sg) % 2 == 0 else nc.gpsimd
                    eng.dma_start(out=x_sb[:, t, sg * 512:(sg + 1) * 512],
                                  in_=x_d[it, t * 128:(t + 1) * 128, sg * 512:(sg + 1) * 512])

        def emit_gn(it):
            """Batched GroupNorm: one combine chain for all 4 c-tiles.
            Writes h in fp8 (QKV DoubleRow rhs)."""
            x_sb = x_sbs[it]
            h_sb = hp.tile([128, CT, HW], FP8, name=f"h_{it}", tag="h")
            stats128 = stp.tile([128, 2 * CT], FP32, name=f"s128_{it}", tag="s128")
            mvs = stp.tile([128, CT, 2], FP32, name=f"mvs_{it}", tag="mvs")
            for t in range(CT):
                bnst = stp.tile([128, NSUB, SGD], FP32, name=f"bnst_{it}_{t}", tag="bnst", bufs=2)
                for sg in range(NSUB):
                    nc.vector.bn_stats(out=bnst[:, sg, :], in_=x_sb[:, t, sg * 512:(sg + 1) * 512])
                nc.vector.bn_aggr(out=mvs[:, t, :], in_=bnst)
            s1v = stats128.rearrange("p (t two) -> p t two", two=2)
            nc.vector.tensor_copy(out=s1v[:, :, 0], in_=mvs[:, :, 0])
            nc.vector.tensor_mul(out=s1v[:, :, 1], in0=mvs[:, :, 0], in1=mvs[:, :, 0])
            nc.vector.tensor_add(out=s1v[:, :, 1], in0=s1v[:, :, 1], in1=mvs[:, :, 1])
            if it == 0:
                for _ in range(4):
                    nc.tensor.matmul(out=warm_ps, lhsT=qkvw_sb[:, 0, 0:128],
                                     rhs=qkvw_sb[:, 0, 0:NHALF], start=True, stop=True)
            gst_ps = psp.tile([8, 2 * CT], FP32, name=f"gstp_{it}", tag="st")
            nc.tensor.matmul(out=gst_ps, lhsT=inda_sb, rhs=stats128, start=True, stop=True)
            gst_sb = stp.tile([8, 2 * CT], FP32, name=f"gst_{it}", tag="gst")
            nc.vector.tensor_copy(out=gst_sb, in_=gst_ps)
            gv = gst_sb.rearrange("p (t two) -> p t two", two=2)
            gm2 = stp.tile([8, CT], FP32, name=f"gm2_{it}", tag="gm2")
            nc.vector.tensor_mul(out=gm2, in0=gv[:, :, 0], in1=gv[:, :, 0])
            nc.vector.tensor_tensor(out=gm2, in0=gv[:, :, 1], in1=gm2, op=ALU.subtract)
            nc.scalar.activation(out=gm2, in_=gm2, func=AF.Sqrt,
                                 bias=eps_sb[0:8, 0:1], scale=1.0)
            nc.vector.reciprocal(out=gv[:, :, 1], in_=gm2)
            chst_ps = psp.tile([128, 2 * CT], FP32, name=f"chstp_{it}", tag="st")
            nc.tensor.matmul(out=chst_ps, lhsT=indb_sb, rhs=gst_sb, start=True, stop=True)
            chst_sb = stp.tile([128, 2 * CT], FP32, name=f"chst_{it}", tag="chst")
            nc.vector.tensor_copy(out=chst_sb, in_=chst_ps)
            cv = chst_sb.rearrange("p (t two) -> p t two", two=2)
            sb2 = stp.tile([128, 2, CT], FP32, name=f"sb2_{it}", tag="sb2")
            nc.vector.tensor_mul(out=sb2[:, 0, :], in0=cv[:, :, 1], in1=gnw_sb)
            nc.vector.tensor_mul(out=sb2[:, 1, :], in0=cv[:, :, 0], in1=sb2[:, 0, :])
            nc.vector.tensor_tensor(out=sb2[:, 1, :], in0=gnb_sb, in1=sb2[:, 1, :], op=ALU.subtract)
            if it == 0:
                # keep the PE warm while it waits on the affine-param chain
                warm2 = psp.tile([128, NHALF], FP32, name="warm2_ps", tag="st")
                for _ in range(4):
                    nc.tensor.matmul(out=warm2, lhsT=qkvw_sb[:, 0, 0:128],
                                     rhs=qkvw_sb[:, 0, 0:NHALF], start=True, stop=True)
            # item 0's apply is on the startup critical path -> fast DVE;
            # item 1's goes to the otherwise-idle gpsimd engine
            aeng = nc.vector if it == 0 else nc.gpsimd
            for t in range(CT):
                aeng.tensor_scalar(
                    out=h_sb[:, t, :], in0=x_sb[:, t, :],
                    scalar1=sb2[:, 0, t:t + 1], scalar2=sb2[:, 1, t:t + 1],
                    op0=ALU.mult, op1=ALU.add)
            return h_sb

        def emit_qk_one(it, qk_sb, h_sb, ot, evac_act=False):
            """One Q'/K' output tile (=16Q etc.) via fp8 DoubleRow QKV matmul."""
            qk_ps = psp.tile([128, HW], FP32, name=f"qkps_{it}_{ot}", tag="st")
            for j in range(2):           # contraction pairs (stationary reuse over halves)
                for half in range(2):
                    nc.tensor.matmul(
                        out=qk_ps[:, half * NHALF:(half + 1) * NHALF],
                        lhsT=qkvw_sb[:, 2 * j:2 * j + 2, ot * 128:(ot + 1) * 128],
                        rhs=h_sb[:, 2 * j:2 * j + 2, half * NHALF:(half + 1) * NHALF],
                        start=(j == 0), stop=(j == 1), perf_mode=DR)
            if evac_act:
                nc.scalar.activation(out=qk_sb[:, ot, :], in_=qk_ps,
                                     func=AF.Identity,
                                     bias=qkvb_sb[:, ot:ot + 1], scale=1.0)
            else:
                nc.vector.tensor_scalar_add(
                    out=qk_sb[:, ot, :], in0=qk_ps, scalar1=qkvb_sb[:, ot:ot + 1])

        def emit_qk_pair(it, qk_sb, h_sb, hd):
            emit_qk_one(it, qk_sb, h_sb, hd)
            emit_qk_one(it, qk_sb, h_sb, 4 + hd)

        def emit_vt(it, vt_sb, h_sb, mt_pairs):
            """V'^T (=16 V^T) [m, d-all-heads] fp8 via swapped-operand DR matmul.
            Two m-chunks share one PSUM tile so the fp8 cast is one DVE op."""
            for mp in mt_pairs:
                vt_ps = psp.tile([128, 2, C], FP32, name=f"vtps_{it}_{mp}", tag="st")
                for half in range(2):
                    mt = 2 * mp + half
                    for j in range(2):
                        nc.tensor.matmul(
                            out=vt_ps[:, half, :],
                            lhsT=h_sb[:, 2 * j:2 * j + 2, mt * 128:(mt + 1) * 128],
                            rhs=qkvw_sb[:, 2 * j:2 * j + 2, 2 * C:3 * C],
                            start=(j == 0), stop=(j == 1), perf_mode=DR)
                nc.vector.tensor_copy(out=vt_sb[:, 2 * mp:2 * mp + 2, :], in_=vt_ps)

        def emit_proj_ot(it, o_sb, ot, via_act=False):
            pr_ps = psp.tile([128, HW], FP32, name=f"prps_{it}_{ot}", tag="st")
            for j in range(2):
                for half in range(2):
                    nc.tensor.matmul(
                        out=pr_ps[:, half * NHALF:(half + 1) * NHALF],
                        lhsT=projw_sb[:, 2 * j:2 * j + 2, ot * 128:(ot + 1) * 128],
                        rhs=o_sb[:, 2 * j:2 * j + 2, half * NHALF:(half + 1) * NHALF],
                        start=(j == 0), stop=(j == 1), perf_mode=DR)
            out_sb = oup.tile([128, HW], FP32, name=f"out_{it}_{ot}", tag="outsb")
            if via_act:
                # kernel tail: ACT is done with exp, DVE is the bottleneck there
                tmp_sb = oup.tile([128, HW], FP32, name=f"tmp_{it}_{ot}", tag="tmpsb", bufs=2)
                nc.scalar.activation(out=tmp_sb, in_=pr_ps, func=AF.Identity,
                                     bias=projb_sb[:, ot:ot + 1], scale=1.0 / (WS * WS))
                nc.gpsimd.tensor_add(out=out_sb, in0=tmp_sb, in1=x_sbs[it][:, ot, :])
            else:
                nc.vector.affine_then_add(
                    out=out_sb, in0=pr_ps, in1=x_sbs[it][:, ot, :],
                    scale=1.0 / (WS * WS), bias=projb_sb[:, ot:ot + 1])
            (nc.sync if ot % 2 == 0 else nc.gpsimd).dma_start(
                out=out_d[it, ot * 128:(ot + 1) * 128, :], in_=out_sb)

        # ---------- 8-unit (item, head) software pipeline ----------
        # Unit u does its own QK^T + exp; the denominator/PV matmuls of unit
        # u-1 are interleaved into u's QK chunk gaps (PE is in-order, so the
        # delayed pieces are wait-free fillers while exp paces the st bufs).
        # flush(u-2) (reciprocal + O = U/denom) is emitted at unit start.
        units = [(it, hd) for it in range(ITEMS) for hd in range(HEADS)]
        U = len(units)
        eTs, dbs, us = {}, {}, {}
        h_sbs = {}
        qk_sbs = {0: qkp.tile([128, 8, HW], FP16, name="qk_0", tag="qk"),
                  1: qkp.tile([128, 8, HW], FP16, name="qk_1", tag="qk")}
        vt_sbs = {0: vtp.tile([128, MT, C], FP8, name="vt_0", tag="vt"),
                  1: vtp.tile([128, MT, C], FP8, name="vt_1", tag="vt")}
        o_sbs = {0: oap.tile([128, CT, HW], FP8, name="o_0", tag="o"),
                 1: oap.tile([128, CT, HW], FP8, name="o_1", tag="o")}

        def flush(u):
            it_f, hd_f = units[u]
            rinv = rip.tile([128, HW], FP32, name=f"rinv_{u}", tag="rinv")
            nc.vector.reciprocal_approx_fast(out=rinv, in_=dbs[u])
            nc.vector.tensor_mul(out=o_sbs[it_f][:, hd_f, :], in0=us[u], in1=rinv)

        def db_piece(u, j):
            for half in range(2):
                nc.tensor.matmul(
                    out=dbs[u][:, half * NHALF:(half + 1) * NHALF],
                    lhsT=ones_sb,
                    rhs=eTs[u][:, 2 * j:2 * j + 2, half * NHALF:(half + 1) * NHALF],
                    start=(j == 0), stop=(j == MT // 2 - 1), perf_mode=DR)

        def pv_piece(u, j):
            it_p, hd_p = units[u]
            for half in range(2):
                nc.tensor.matmul(
                    out=us[u][:, half * NHALF:(half + 1) * NHALF],
                    lhsT=vt_sbs[it_p][:, 2 * j:2 * j + 2, hd_p * 128:(hd_p + 1) * 128],
                    rhs=eTs[u][:, 2 * j:2 * j + 2, half * NHALF:(half + 1) * NHALF],
                    start=(j == 0), stop=(j == MT // 2 - 1), perf_mode=DR)

        def alloc_ps(u):
            dbs[u] = psp.tile([128, HW], FP32, name=f"dbps_{u}", tag="db", bufs=1)
            us[u] = psp.tile([128, HW], FP32, name=f"ups_{u}", tag="u", bufs=1)

        # startup: GN(0), first head's Q/K only, then straight into the pipeline
        h_sbs[0] = emit_gn(0)
        emit_qk_pair(0, qk_sbs[0], h_sbs[0], 0)
        emit_x_dma(1)

        def qk1f(it, ot, evac_act=False):
            return lambda: emit_qk_one(it, qk_sbs[it], h_sbs[it], ot, evac_act)

        def vtf(it, mp):
            return lambda: emit_vt(it, vt_sbs[it], h_sbs[it], [mp])

        def gnf(it):
            def f():
                h_sbs[it] = emit_gn(it)
            return f

        def projf(it, ot):
            return lambda: emit_proj_ot(it, o_sbs[it], ot)

        # a few psum evacuations ride on ACT (its exp stream has slack) to
        # relieve DVE, which is the serializer in the first half of the kernel
        fillers = {
            0: [qk1f(0, 1), qk1f(0, 5), vtf(0, 0), qk1f(0, 2), qk1f(0, 6, True),
                vtf(0, 1), qk1f(0, 3), qk1f(0, 7, True), vtf(0, 2), vtf(0, 3)],
            1: [gnf(1)],
            2: [qk1f(1, 0), qk1f(1, 4, True), qk1f(1, 1), qk1f(1, 5)],
            3: [qk1f(1, 2), qk1f(1, 6, True), qk1f(1, 3), qk1f(1, 7), vtf(1, 0)],
            4: [vtf(1, 1), vtf(1, 2), vtf(1, 3)],
            5: [projf(0, 0)],
            6: [projf(0, 1)],
            7: [projf(0, 2), projf(0, 3)],
        }

        for u, (it_u, hd_u) in enumerate(units):
            last = (u == U - 1)
            if u >= 2 and u < U - 1:
                flush(u - 2)
            if last:
                flush(U - 2)       # U-2 self-drained at the end of unit U-2
                alloc_ps(u)        # own psums for the eager in-gap pieces
            fill = []
            if u >= 1 and u < U - 1:
                alloc_ps(u - 1)
                for j in range(MT // 2):
                    fill.append(lambda u=u - 1, j=j: db_piece(u, j))
                    fill.append(lambda u=u - 1, j=j: pv_piece(u, j))
            fill.extend(fillers.get(u, []))
            eT = etp.tile([128, MT, HW], FP8, name=f"eT_{u}", tag="eT")
            eTs[u] = eT
            qk_sb = qk_sbs[it_u]
            for mt in range(MT):
                st_ps = psp.tile([128, HW], FP32, name=f"stps_{u}_{mt}", tag="st")
                for half in range(2):
                    nc.tensor.matmul(
                        out=st_ps[:, half * NHALF:(half + 1) * NHALF],
                        lhsT=qk_sb[:, 4 + hd_u, mt * 128:(mt + 1) * 128],
                        rhs=qk_sb[:, hd_u, half * NHALF:(half + 1) * NHALF],
                        start=True, stop=True)
                nc.scalar.activation(out=eT[:, mt, :], in_=st_ps,
                                     func=AF.Exp, bias=ebias_sb,
                                     scale=SCALE / (WS * WS))
                # hold back >=2 fill pieces for the unit boundary (the next
                # unit's first QK chunks wait on this unit's exp via st bufs)
                if mt >= 2 and fill:
                    fill.pop(0)()
                if last and mt >= 5:
                    db_piece(u, mt - 5)
                    pv_piece(u, mt - 5)
            for f in fill:
                f()
            if u == U - 2:
                # self-drain: own denominator/PV now so unit U-1 can flush us
                # at its start and run its own pieces eagerly in-gap
                flush(u - 1)
                alloc_ps(u)
                for j in range(MT // 2):
                    db_piece(u, j)
                    pv_piece(u, j)
            if last:
                db_piece(u, MT // 2 - 1)
                pv_piece(u, MT // 2 - 1)

        flush(U - 1)
        for ot in range(CT):
            emit_proj_ot(1, o_sbs[1], ot)

    nc.compile()
    return nc


def _host_prep(x, gn_w, gn_b, qkv_w, qkv_b, proj_w, proj_b):
    x = np.ascontiguousarray(np.asarray(x, dtype=np.float32)).reshape(B, C, HW)
    qkv_w = np.asarray(qkv_w, dtype=np.float32)
    qkv_b = np.asarray(qkv_b, dtype=np.float32)
    proj_w = np.asarray(proj_w, dtype=np.float32)
    proj_b = np.asarray(proj_b, dtype=np.float32)
    gn_w = np.asarray(gn_w, dtype=np.float32)
    gn_b = np.asarray(gn_b, dtype=np.float32)

    qkv_wT = np.ascontiguousarray(qkv_w.T * WS).astype(NP8)            # [C, 3C]
    proj_wT = np.ascontiguousarray(proj_w.T * WS).astype(NP8)          # [C, C]
    # q,k evac adds 16*bias (Q' = 16Q + 16 b_q)
    qkv_bt = np.ascontiguousarray((WS * qkv_b[:2 * C]).reshape(8, 128).T)  # [128, 8]
    proj_be = proj_b + proj_w @ qkv_b[2 * C:]                          # fold V bias
    proj_bt = np.ascontiguousarray(proj_be.reshape(CT, 128).T)         # [128, CT]
    gn_wt = np.ascontiguousarray(gn_w.reshape(CT, 128).T)
    gn_bt = np.ascontiguousarray(gn_b.reshape(CT, 128).T)

    p = np.arange(128)
    ind_a = np.zeros((128, 8), np.float32)
    ind_a[p, p // 16] = 1.0 / 16.0
    ind_b = np.zeros((8, 128), np.float32)
    ind_b[p // 16, p] = 1.0

    shared = {
        "qkv_wT": qkv_wT, "qkv_bt": qkv_bt.astype(np.float32),
        "proj_wT": proj_wT, "proj_bt": proj_bt.astype(np.float32),
        "gn_wt": gn_wt.astype(np.float32), "gn_bt": gn_bt.astype(np.float32),
        "ind_a": ind_a, "ind_b": ind_b,
    }
    in_maps = []
    for c in range(N_CORES):
        m = dict(shared)
        m["x"] = np.ascontiguousarray(x[c * ITEMS:(c + 1) * ITEMS])
        in_maps.append(m)
    return in_maps


def kernel(x, gn_w, gn_b, qkv_w, qkv_b, proj_w, proj_b):
    if "nc" not in _CACHE:
        _CACHE["nc"] = _build_bass()
    nc = _CACHE["nc"]
    in_maps = _host_prep(x, gn_w, gn_b, qkv_w, qkv_b, proj_w, proj_b)
    trace = bool(int(os.environ.get("KERNEL_TRACE", "0")))
    res = bass_utils.run_bass_kernel_spmd(
        nc, in_maps, core_ids=list(range(N_CORES)), trace=trace)
    _CACHE["last_results"] = res
    out = np.concatenate([r["out"] for r in res.results], axis=0)
    return out.reshape(B, C, 32, 32).astype(np.float32)


# revision 29
# speedup vs baseline: 1.0291x; 1.0213x over previous
"""Trainium2 Bass kernel for a GroupNorm->QKV->MHA->proj->residual block.

Problem shapes (hardcoded): x [16, 512, 32, 32] fp32, 4 heads, head_dim 128,
32 GN groups. Sharding: data-parallel over batch across 8 NeuronCores
(2 items per core), no collectives.

v2: fp8 (TRN float8e4, max 240) DoubleRow matmuls for QKV / PV / softmax
denominator / proj (2x PE rate via 256-deep contraction pairs); QK^T stays
fp16 (d=128 contraction cannot use DoubleRow). Weights are scaled x16 on
host so fp8 quantization stays out of the subnormal range; the scale is
repaid inside the exp activation (scale=SCALE/256) and the final residual
affine (1/256). The softmax denominator is computed on the PE with an fp8
ones-matmul over the exp chunks (replaces ~36us of DVE tree adds).

Layout per core, per item:
  x      [c=512, n=1024]  as 4 c-tiles of [128, 1024] fp32
  h8     = GN(x) in fp8, [128, CT, HW] (QKV DoubleRow rhs)
  Q',K'  [d=128, n=1024] fp16 per head, = 16*(Q,K) (plain psum copy evac)
  V'^T   [m=1024, d=512] fp8, = 16*V^T via swapped-operand DR matmul
  S'     = K'^T Q' fp16 matmul = 256*S; exp scale folds 1/256
  E^T    = exp(S*scale - 1.5) fp8 [128, MT, HW] (max ~134 < 240)
  denom  = ones8-DR-matmul over E^T chunks, accumulated in PSUM [128, HW]
  U'     = V'^T.T @ E^T fp8-DR in PSUM; O' = U' * (1/denom) in fp8 (=16*O)
  proj   fp8-DR PSUM matmul (=256*proj), then one DVE affine_then_add:
         out = (psum*(1/256) + proj_b') + x
  proj_b' (host) = proj_b + proj_w @ qkv_b[1024:]  (folds the V bias exactly)
"""

import os
import numpy as np
import ml_dtypes
from contextlib import ExitStack

from concourse import bass, bacc, mybir
import concourse.tile as tile
from concourse import bass_utils

FP32 = mybir.dt.float32
FP16 = mybir.dt.float16
FP8 = mybir.dt.float8e4
NP8 = ml_dtypes.float8_e4m3

N_CORES = 8
B = 16
ITEMS = B // N_CORES      # batch items per core
C = 512
HW = 1024                 # H*W
HEADS = 4
D = 128                   # head dim
CT = C // 128             # channel tiles
MT = HW // 128            # m-chunks of the softmax axis
NHALF = 512               # matmul free-dim half (one PSUM bank)
SCALE = float(D) ** -0.5
WS = 16.0                 # host weight scale (fp8 subnormal dodge)
EXP_BIAS = -1.5
EPS = 1e-5
DR = mybir.MatmulPerfMode.DoubleRow

_CACHE = {}


def _build_bass():
    nc = bacc.Bacc()

    x_d = nc.declare_dram_parameter("x", [ITEMS, C, HW], FP32, isOutput=False)
    qkvw_d = nc.declare_dram_parameter("qkv_wT", [C, 3 * C], FP8, isOutput=False)
    qkvb_d = nc.declare_dram_parameter("qkv_bt", [128, 8], FP32, isOutput=False)
    projw_d = nc.declare_dram_parameter("proj_wT", [C, C], FP8, isOutput=False)
    projb_d = nc.declare_dram_parameter("proj_bt", [128, CT], FP32, isOutput=False)
    gnw_d = nc.declare_dram_parameter("gn_wt", [128, CT], FP32, isOutput=False)
    gnb_d = nc.declare_dram_parameter("gn_bt", [128, CT], FP32, isOutput=False)
    inda_d = nc.declare_dram_parameter("ind_a", [128, 8], FP32, isOutput=False)
    indb_d = nc.declare_dram_parameter("ind_b", [8, 128], FP32, isOutput=False)
    out_d = nc.declare_dram_parameter("out", [ITEMS, C, HW], FP32, isOutput=True)

    AF = mybir.ActivationFunctionType
    ALU = mybir.AluOpType

    with ExitStack() as ctx:
        tc = ctx.enter_context(tile.TileContext(nc))
        singles = ctx.enter_context(tc.tile_pool(name="singles", bufs=1))
        xp = ctx.enter_context(tc.tile_pool(name="xp", bufs=2))
        hp = ctx.enter_context(tc.tile_pool(name="hp", bufs=2))
        qkp = ctx.enter_context(tc.tile_pool(name="qkp", bufs=2))
        vtp = ctx.enter_context(tc.tile_pool(name="vtp", bufs=2))
        etp = ctx.enter_context(tc.tile_pool(name="etp", bufs=2))
        rip = ctx.enter_context(tc.tile_pool(name="rip", bufs=2))
        oap = ctx.enter_context(tc.tile_pool(name="oap", bufs=2))
        oup = ctx.enter_context(tc.tile_pool(name="oup", bufs=2))
        stp = ctx.enter_context(tc.tile_pool(name="stp", bufs=4))
        psp = ctx.enter_context(tc.tile_pool(name="psp", bufs=2, space="PSUM"))

        # ---- item-0 x first (startup critical path), then weights ----
        # x on 3 round-robin DMA queues in small chunks (fast first-landing
        # for the GroupNorm stats chain); weights on the scalar engine's
        # queue so they don't serialize behind x
        XQ = (nc.sync, nc.gpsimd)
        x_sbs = {}
        x_sbs[0] = xp.tile([128, CT, HW], FP32, name="x_0", tag="x")
        for t in range(CT):
            for sg in range(4):
                XQ[(4 * t + sg) % 2].dma_start(
                    out=x_sbs[0][:, t, sg * 256:(sg + 1) * 256],
                    in_=x_d[0, t * 128:(t + 1) * 128, sg * 256:(sg + 1) * 256])

        qkvw_sb = singles.tile([128, CT, 3 * C], FP8)
        for t in range(CT):
            nc.scalar.dma_start(out=qkvw_sb[:, t, :], in_=qkvw_d[t * 128:(t + 1) * 128, :])
        # keep PE busy/warm during the GroupNorm startup window (HAM clock gate)
        warm_ps = psp.tile([128, NHALF], FP32, name="warm_ps", tag="st")
        for _ in range(16):
            nc.tensor.matmul(out=warm_ps, lhsT=qkvw_sb[:, 0, 0:128],
                             rhs=qkvw_sb[:, 0, 0:NHALF], start=True, stop=True)
        projw_sb = singles.tile([128, CT, C], FP8)
        for t in range(CT):
            nc.scalar.dma_start(out=projw_sb[:, t, :], in_=projw_d[t * 128:(t + 1) * 128, :])
        qkvb_sb = singles.tile([128, 8], FP32)
        nc.sync.dma_start(out=qkvb_sb, in_=qkvb_d[:, :])
        projb_sb = singles.tile([128, CT], FP32)
        nc.sync.dma_start(out=projb_sb, in_=projb_d[:, :])
        gnw_sb = singles.tile([128, CT], FP32)
        nc.gpsimd.dma_start(out=gnw_sb, in_=gnw_d[:, :])
        gnb_sb = singles.tile([128, CT], FP32)
        nc.gpsimd.dma_start(out=gnb_sb, in_=gnb_d[:, :])
        inda_sb = singles.tile([128, 8], FP32)
        nc.sync.dma_start(out=inda_sb, in_=inda_d[:, :])
        indb_sb = singles.tile([8, 128], FP32)
        nc.gpsimd.dma_start(out=indb_sb, in_=indb_d[:, :])
        ones_sb = singles.tile([128, 2, 128], FP8)
        nc.vector.memset(ones_sb, 1.0)
        eps_sb = singles.tile([128, 1], FP32)
        nc.vector.memset(eps_sb, EPS)
        ebias_sb = singles.tile([128, 1], FP32)
        nc.vector.memset(ebias_sb, EXP_BIAS)
        warm_sb = singles.tile([128, 1], FP32)
        nc.vector.memset(warm_sb, 1.0)
        nc.scalar.activation(out=warm_sb, in_=warm_sb, func=AF.Sqrt,
                             bias=eps_sb[:, 0:1], scale=1.0)

        SGD = nc.vector.BN_STATS_DIM   # 6
        NSUB = HW // nc.vector.BN_STATS_FMAX  # 2 subgroups of 512

        def emit_x_dma(it):
            x_sb = xp.tile([128, CT, HW], FP32, name=f"x_{it}", tag="x")
            x_sbs[it] = x_sb
            for t in range(CT):
                for sg in range(2):
                    XQ[(2 * t + sg) % 2].dma_start(
                        out=x_sb[:, t, sg * 512:(sg + 1) * 512],
                        in_=x_d[it, t * 128:(t + 1) * 128, sg * 512:(sg + 1) * 512])

        def emit_gn(it):
            """Batched GroupNorm: one combine chain for all 4 c-tiles.
            Writes h in fp8 (QKV DoubleRow rhs). For item 0 (startup critical
            path) half the per-partition stats run on ACT via accum_out."""
            x_sb = x_sbs[it]
            h_sb = hp.tile([128, CT, HW], FP8, name=f"h_{it}", tag="h")
            stats128 = stp.tile([128, 2 * CT], FP32, name=f"s128_{it}", tag="s128")
            s1v = stats128.rearrange("p (t two) -> p t two", two=2)
            dve_tiles = range(2) if it == 0 else range(CT)
            act_tiles = range(2, CT) if it == 0 else ()
            nt = len(dve_tiles)
            mvs = stp.tile([128, nt, 2], FP32, name=f"mvs_{it}", tag="mvs")
            for i, t in enumerate(dve_tiles):
                bnst = stp.tile([128, NSUB, SGD], FP32, name=f"bnst_{it}_{t}", tag="bnst", bufs=2)
                for sg in range(NSUB):
                    nc.vector.bn_stats(out=bnst[:, sg, :], in_=x_sb[:, t, sg * 512:(sg + 1) * 512])
                nc.vector.bn_aggr(out=mvs[:, i, :], in_=bnst)
            if act_tiles:
                acc = stp.tile([128, CT, 2], FP32, name=f"acc_{it}", tag="acc")
                scr = stp.tile([128, HW], FP32, name=f"scr_{it}", tag="scr")
                for t in act_tiles:
                    nc.scalar.activation(out=scr, in_=x_sb[:, t, :], func=AF.Square,
                                         accum_out=acc[:, t, 0:1])
                    nc.scalar.activation(out=scr, in_=x_sb[:, t, :], func=AF.Identity,
                                         bias=0.0, accum_out=acc[:, t, 1:2])
                for t in act_tiles:
                    nc.vector.tensor_scalar_mul(out=s1v[:, t, 0:1], in0=acc[:, t, 1:2],
                                                scalar1=1.0 / HW)
                    nc.vector.tensor_scalar_mul(out=s1v[:, t, 1:2], in0=acc[:, t, 0:1],
                                                scalar1=1.0 / HW)
            dts = slice(0, nt)
            nc.vector.tensor_copy(out=s1v[:, dts, 0], in_=mvs[:, :, 0])
            nc.vector.tensor_mul(out=s1v[:, dts, 1], in0=mvs[:, :, 0], in1=mvs[:, :, 0])
            nc.vector.tensor_add(out=s1v[:, dts, 1], in0=s1v[:, dts, 1], in1=mvs[:, :, 1])
            gst_ps = psp.tile([8, 2 * CT], FP32, name=f"gstp_{it}", tag="st")
            nc.tensor.matmul(out=gst_ps, lhsT=inda_sb, rhs=stats128, start=True, stop=True)
            gst_sb = stp.tile([8, 2 * CT], FP32, name=f"gst_{it}", tag="gst")
            nc.vector.tensor_copy(out=gst_sb, in_=gst_ps)
            gv = gst_sb.rearrange("p (t two) -> p t two", two=2)
            gm2 = stp.tile([8, CT], FP32, name=f"gm2_{it}", tag="gm2")
            nc.vector.tensor_mul(out=gm2, in0=gv[:, :, 0], in1=gv[:, :, 0])
            nc.vector.tensor_tensor(out=gm2, in0=gv[:, :, 1], in1=gm2, op=ALU.subtract)
            nc.scalar.activation(out=gm2, in_=gm2, func=AF.Sqrt,
                                 bias=eps_sb[0:8, 0:1], scale=1.0)
            nc.vector.reciprocal(out=gv[:, :, 1], in_=gm2)
            chst_ps = psp.tile([128, 2 * CT], FP32, name=f"chstp_{it}", tag="st")
            nc.tensor.matmul(out=chst_ps, lhsT=indb_sb, rhs=gst_sb, start=True, stop=True)
            chst_sb = stp.tile([128, 2 * CT], FP32, name=f"chst_{it}", tag="chst")
            nc.vector.tensor_copy(out=chst_sb, in_=chst_ps)
            cv = chst_sb.rearrange("p (t two) -> p t two", two=2)
            sb2 = stp.tile([128, 2, CT], FP32, name=f"sb2_{it}", tag="sb2")
            nc.vector.tensor_mul(out=sb2[:, 0, :], in0=cv[:, :, 1], in1=gnw_sb)
            nc.vector.tensor_mul(out=sb2[:, 1, :], in0=cv[:, :, 0], in1=sb2[:, 0, :])
            nc.vector.tensor_tensor(out=sb2[:, 1, :], in0=gnb_sb, in1=sb2[:, 1, :], op=ALU.subtract)

            # item 0's apply is on the startup critical path -> fast DVE;
            # item 1's goes to the otherwise-idle gpsimd engine
            aeng = nc.vector if it == 0 else nc.gpsimd
            for t in range(CT):
                aeng.tensor_scalar(
                    out=h_sb[:, t, :], in0=x_sb[:, t, :],
                    scalar1=sb2[:, 0, t:t + 1], scalar2=sb2[:, 1, t:t + 1],
                    op0=ALU.mult, op1=ALU.add)
            return h_sb

        def emit_qk_one(it, qk_sb, h_sb, ot, evac_act=False):
            """One Q'/K' output tile (=16Q etc.) via fp8 DoubleRow QKV matmul."""
            qk_ps = psp.tile([128, HW], FP32, name=f"qkps_{it}_{ot}", tag="st")
            for j in range(2):           # contraction pairs (stationary reuse over halves)
                for half in range(2):
                    nc.tensor.matmul(
                        out=qk_ps[:, half * NHALF:(half + 1) * NHALF],
                        lhsT=qkvw_sb[:, 2 * j:2 * j + 2, ot * 128:(ot + 1) * 128],
                        rhs=h_sb[:, 2 * j:2 * j + 2, half * NHALF:(half + 1) * NHALF],
                        start=(j == 0), stop=(j == 1), perf_mode=DR)
            if evac_act:
                nc.scalar.activation(out=qk_sb[:, ot, :], in_=qk_ps,
                                     func=AF.Identity,
                                     bias=qkvb_sb[:, ot:ot + 1], scale=1.0)
            else:
                nc.vector.tensor_scalar_add(
                    out=qk_sb[:, ot, :], in0=qk_ps, scalar1=qkvb_sb[:, ot:ot + 1])

        def emit_qk_pair(it, qk_sb, h_sb, hd):
            emit_qk_one(it, qk_sb, h_sb, hd)
            emit_qk_one(it, qk_sb, h_sb, 4 + hd)

        def emit_vt(it, vt_sb, h_sb, mt_pairs):
            """V'^T (=16 V^T) [m, d-all-heads] fp8 via swapped-operand DR matmul.
            Two m-chunks share one PSUM tile so the fp8 cast is one DVE op."""
            for mp in mt_pairs:
                vt_ps = psp.tile([128, 2, C], FP32, name=f"vtps_{it}_{mp}", tag="st")
                for half in range(2):
                    mt = 2 * mp + half
                    for j in range(2):
                        nc.tensor.matmul(
                            out=vt_ps[:, half, :],
                            lhsT=h_sb[:, 2 * j:2 * j + 2, mt * 128:(mt + 1) * 128],
                            rhs=qkvw_sb[:, 2 * j:2 * j + 2, 2 * C:3 * C],
                            start=(j == 0), stop=(j == 1), perf_mode=DR)
                nc.vector.tensor_copy(out=vt_sb[:, 2 * mp:2 * mp + 2, :], in_=vt_ps)

        def emit_proj_ot(it, o_sb, ot, via_act=False):
            pr_ps = psp.tile([128, HW], FP32, name=f"prps_{it}_{ot}", tag="st")
            for j in range(2):
                for half in range(2):
                    nc.tensor.matmul(
                        out=pr_ps[:, half * NHALF:(half + 1) * NHALF],
                        lhsT=projw_sb[:, 2 * j:2 * j + 2, ot * 128:(ot + 1) * 128],
                        rhs=o_sb[:, 2 * j:2 * j + 2, half * NHALF:(half + 1) * NHALF],
                        start=(j == 0), stop=(j == 1), perf_mode=DR)
            out_sb = oup.tile([128, HW], FP32, name=f"out_{it}_{ot}", tag="outsb")
            if via_act:
                # kernel tail: ACT is done with exp, DVE is the bottleneck there
                tmp_sb = oup.tile([128, HW], FP32, name=f"tmp_{it}_{ot}", tag="tmpsb", bufs=2)
                nc.scalar.activation(out=tmp_sb, in_=pr_ps, func=AF.Identity,
                                     bias=projb_sb[:, ot:ot + 1], scale=1.0 / (WS * WS))
                nc.gpsimd.tensor_add(out=out_sb, in0=tmp_sb, in1=x_sbs[it][:, ot, :])
            else:
                nc.vector.affine_then_add(
                    out=out_sb, in0=pr_ps, in1=x_sbs[it][:, ot, :],
                    scale=1.0 / (WS * WS), bias=projb_sb[:, ot:ot + 1])
            (nc.sync if ot % 2 == 0 else nc.gpsimd).dma_start(
                out=out_d[it, ot * 128:(ot + 1) * 128, :], in_=out_sb)

        # ---------- 8-unit (item, head) software pipeline ----------
        # Unit u does its own QK^T + exp; the denominator/PV matmuls of unit
        # u-1 are interleaved into u's QK chunk gaps (PE is in-order, so the
        # delayed pieces are wait-free fillers while exp paces the st bufs).
        # flush(u-2) (reciprocal + O = U/denom) is emitted at unit start.
        units = [(it, hd) for it in range(ITEMS) for hd in range(HEADS)]
        U = len(units)
        eTs, dbs, us = {}, {}, {}
        h_sbs = {}
        qk_sbs = {0: qkp.tile([128, 8, HW], FP16, name="qk_0", tag="qk"),
                  1: qkp.tile([128, 8, HW], FP16, name="qk_1", tag="qk")}
        vt_sbs = {0: vtp.tile([128, MT, C], FP8, name="vt_0", tag="vt"),
                  1: vtp.tile([128, MT, C], FP8, name="vt_1", tag="vt")}
        o_sbs = {0: oap.tile([128, CT, HW], FP8, name="o_0", tag="o"),
                 1: oap.tile([128, CT, HW], FP8, name="o_1", tag="o")}

        def flush(u):
            it_f, hd_f = units[u]
            rinv = rip.tile([128, HW], FP32, name=f"rinv_{u}", tag="rinv")
            nc.vector.reciprocal_approx_fast(out=rinv, in_=dbs[u])
            nc.vector.tensor_mul(out=o_sbs[it_f][:, hd_f, :], in0=us[u], in1=rinv)

        def db_piece(u, j):
            for half in range(2):
                nc.tensor.matmul(
                    out=dbs[u][:, half * NHALF:(half + 1) * NHALF],
                    lhsT=ones_sb,
                    rhs=eTs[u][:, 2 * j:2 * j + 2, half * NHALF:(half + 1) * NHALF],
                    start=(j == 0), stop=(j == MT // 2 - 1), perf_mode=DR)

        def pv_piece(u, j):
            it_p, hd_p = units[u]
            for half in range(2):
                nc.tensor.matmul(
                    out=us[u][:, half * NHALF:(half + 1) * NHALF],
                    lhsT=vt_sbs[it_p][:, 2 * j:2 * j + 2, hd_p * 128:(hd_p + 1) * 128],
                    rhs=eTs[u][:, 2 * j:2 * j + 2, half * NHALF:(half + 1) * NHALF],
                    start=(j == 0), stop=(j == MT // 2 - 1), perf_mode=DR)

        def alloc_ps(u):
            dbs[u] = psp.tile([128, HW], FP32, name=f"dbps_{u}", tag="db", bufs=1)
            us[u] = psp.tile([128, HW], FP32, name=f"ups_{u}", tag="u", bufs=1)

        # startup: GN(0), first head's Q/K only, then straight into the pipeline
        h_sbs[0] = emit_gn(0)
        emit_qk_pair(0, qk_sbs[0], h_sbs[0], 0)
        emit_x_dma(1)

        def qk1f(it, ot, evac_act=False):
            return lambda: emit_qk_one(it, qk_sbs[it], h_sbs[it], ot, evac_act)

        def vtf(it, mp):
            return lambda: emit_vt(it, vt_sbs[it], h_sbs[it], [mp])

        def gnf(it):
            def f():
                h_sbs[it] = emit_gn(it)
            return f

        def projf(it, ot):
            return lambda: emit_proj_ot(it, o_sbs[it], ot)

        # a few psum evacuations ride on ACT (its exp stream has slack) to
        # relieve DVE, which is the serializer in the first half of the kernel
        fillers = {
            0: [qk1f(0, 1), qk1f(0, 5), vtf(0, 0), qk1f(0, 2), qk1f(0, 6, True),
                vtf(0, 1), qk1f(0, 3), qk1f(0, 7, True), vtf(0, 2), vtf(0, 3)],
            1: [gnf(1)],
            2: [qk1f(1, 0), qk1f(1, 4, True), qk1f(1, 1), qk1f(1, 5)],
            3: [qk1f(1, 2), qk1f(1, 6, True), qk1f(1, 3), qk1f(1, 7), vtf(1, 0)],
            4: [vtf(1, 1), vtf(1, 2), vtf(1, 3)],
            5: [projf(0, 0)],
            6: [projf(0, 1)],
            7: [projf(0, 2), projf(0, 3)],
        }

        for u, (it_u, hd_u) in enumerate(units):
            last = (u == U - 1)
            if u >= 2 and u < U - 1:
                flush(u - 2)
            if last:
                flush(U - 2)       # U-2 self-drained at the end of unit U-2
                alloc_ps(u)        # own psums for the eager in-gap pieces
            fill = []
            if u >= 1 and u < U - 1:
                alloc_ps(u - 1)
                for j in range(MT // 2):
                    fill.append(lambda u=u - 1, j=j: db_piece(u, j))
                    fill.append(lambda u=u - 1, j=j: pv_piece(u, j))
            fill.extend(fillers.get(u, []))
            eT = etp.tile([128, MT, HW], FP8, name=f"eT_{u}", tag="eT", bufs=3)
            eTs[u] = eT
            qk_sb = qk_sbs[it_u]
            for mt in range(MT):
                st_ps = psp.tile([128, HW], FP32, name=f"stps_{u}_{mt}", tag="st")
                for half in range(2):
                    nc.tensor.matmul(
                        out=st_ps[:, half * NHALF:(half + 1) * NHALF],
                        lhsT=qk_sb[:, 4 + hd_u, mt * 128:(mt + 1) * 128],
                        rhs=qk_sb[:, hd_u, half * NHALF:(half + 1) * NHALF],
                        start=True, stop=True)
                nc.scalar.activation(out=eT[:, mt, :], in_=st_ps,
                                     func=AF.Exp, bias=ebias_sb,
                                     scale=SCALE / (WS * WS))
                # hold back >=2 fill pieces for the unit boundary (the next
                # unit's first QK chunks wait on this unit's exp via st bufs)
                if mt >= 2 and fill:
                    fill.pop(0)()
                if last and mt >= 5:
                    db_piece(u, mt - 5)
                    pv_piece(u, mt - 5)
            for f in fill:
                f()
            if u == U - 2:
                # self-drain: own denominator/PV now so unit U-1 can flush us
                # at its start and run its own pieces eagerly in-gap
                flush(u - 1)
                alloc_ps(u)
                for j in range(MT // 2):
                    db_piece(u, j)
                    pv_piece(u, j)
            if last:
                db_piece(u, MT // 2 - 1)
                pv_piece(u, MT // 2 - 1)

        flush(U - 1)
        for ot in range(CT):
            emit_proj_ot(1, o_sbs[1], ot)

    nc.compile()
    return nc


def _host_prep(x, gn_w, gn_b, qkv_w, qkv_b, proj_w, proj_b):
    x = np.ascontiguousarray(np.asarray(x, dtype=np.float32)).reshape(B, C, HW)
    qkv_w = np.asarray(qkv_w, dtype=np.float32)
    qkv_b = np.asarray(qkv_b, dtype=np.float32)
    proj_w = np.asarray(proj_w, dtype=np.float32)
    proj_b = np.asarray(proj_b, dtype=np.float32)
    gn_w = np.asarray(gn_w, dtype=np.float32)
    gn_b = np.asarray(gn_b, dtype=np.float32)

    qkv_wT = np.ascontiguousarray(qkv_w.T * WS).astype(NP8)            # [C, 3C]
    proj_wT = np.ascontiguousarray(proj_w.T * WS).astype(NP8)          # [C, C]
    # q,k evac adds 16*bias (Q' = 16Q + 16 b_q)
    qkv_bt = np.ascontiguousarray((WS * qkv_b[:2 * C]).reshape(8, 128).T)  # [128, 8]
    proj_be = proj_b + proj_w @ qkv_b[2 * C:]                          # fold V bias
    proj_bt = np.ascontiguousarray(proj_be.reshape(CT, 128).T)         # [128, CT]
    gn_wt = np.ascontiguousarray(gn_w.reshape(CT, 128).T)
    gn_bt = np.ascontiguousarray(gn_b.reshape(CT, 128).T)

    p = np.arange(128)
    ind_a = np.zeros((128, 8), np.float32)
    ind_a[p, p // 16] = 1.0 / 16.0
    ind_b = np.zeros((8, 128), np.float32)
    ind_b[p // 16, p] = 1.0

    shared = {
        "qkv_wT": qkv_wT, "qkv_bt": qkv_bt.astype(np.float32),
        "proj_wT": proj_wT, "proj_bt": proj_bt.astype(np.float32),
        "gn_wt": gn_wt.astype(np.float32), "gn_bt": gn_bt.astype(np.float32),
        "ind_a": ind_a, "ind_b": ind_b,
    }
    in_maps = []
    for c in range(N_CORES):
        m = dict(shared)
        m["x"] = np.ascontiguousarray(x[c * ITEMS:(c + 1) * ITEMS])
        in_maps.append(m)
    return in_maps


def kernel(x, gn_w, gn_b, qkv_w, qkv_b, proj_w, proj_b):
    if "nc" not in _CACHE:
        _CACHE["nc"] = _build_bass()
    nc = _CACHE["nc"]
    in_maps = _host_prep(x, gn_w, gn_b, qkv_w, qkv_b, proj_w, proj_b)
    trace = bool(int(os.environ.get("KERNEL_TRACE", "0")))
    res = bass_utils.run_bass_kernel_spmd(
        nc, in_maps, core_ids=list(range(N_CORES)), trace=trace)
    _CACHE["last_results"] = res
    out = np.concatenate([r["out"] for r in res.results], axis=0)
    return out.reshape(B, C, 32, 32).astype(np.float32)
